# revision 64
# baseline (speedup 1.0000x reference)
"""Trainium2 Bass kernel for nn_DarcyResidual (P=256, B=128, 8 NeuronCores).

Math (reference):
    a = (x0 + 1.5) / 0.2,  p = (x1 + 0.9) / 115
    residual = -a*(p_d00 + p_d11) - a_d0*p_d0 - a_d1*p_d1 - 1
2nd-order central differences inside, 2nd-order one-sided at borders,
h = 1/256 on both axes.

Folded form computed here (G = 5/(460 h^2) = 65536/92):
    residual = -G * [ X0'*U4 + S1*R1 + C1a*C1p ] - 1
      X0' = X0 + 1.5  (added on host; all stencil row sums are 0 so the
                       shift does not change any derivative)
      U4  = 4*(rowD2raw(X1) + colD2raw(X1))   (raw h^2-scaled 2nd diffs)
      R1  = rowD1raw(X1), S1 = rowD1raw(X0')  (raw 2h-scaled 1st diffs)
      C1p = colD1raw(X1), C1a = colD1raw(X0')

v3 layout per core (16 images): SBUF [partition = row-within-128-block,
free = (row-block k:2, image b:2, col j:256)], 8 chunks of 2 images.

Engine assignment per chunk:
  PE:   R1 = D1@X1 and S1 = D1@X0' as single fp8 DoubleRow matmuls
        (diag + cross-block k-planes fused, 2x fp8 throughput; fp8
        input error only perturbs the minor grad-product term);
        U4 row part as bf16 WR2 diag matmuls + contract-1 cross fixups;
        U4 col part as 4I matmuls on +-1-shifted bf16 rhs views;
        final merge res = I@tm + I@(t2+t3) into PSUM.
  ACT:  evacuate S1|R1 PSUM -> bf16 (shift-1), final fused
        evacuate+affine(-G,-1), edge-column writes.
  DVE:  C1p/C1a shifted subs (2x), tm = stt(X0'*U4psum) merged [1022],
        t2 = TT(S1b*R1b) (2x), sum1 = t2+t3 (2x).
  GP:   t3 = C1a*C1p, output DMAs (SWDGE).

Border columns j=0,255 keep the f32r edge pipeline fed by a
host-pregathered edge tensor.  Output is bf16 (upcast on host).
"""

import numpy as np

P = 256
B = 128
NCORES = 8
BPC = B // NCORES          # images per core = 16
CHUNKS = 8
BCH = BPC // CHUNKS        # images per chunk = 2
FCH = 2 * BCH * P          # chunk free size = 1024
GAMMA = 5.0 * 65536.0 / 460.0

_cache = {}


def _stencils():
    D1 = np.zeros((P, P), dtype=np.float64)
    for i in range(1, P - 1):
        D1[i, i - 1] = -1.0
        D1[i, i + 1] = 1.0
    D1[0, 0:3] = [-3.0, 4.0, -1.0]
    D1[P - 1, P - 3:P] = [1.0, -4.0, 3.0]

    D2 = np.zeros((P, P), dtype=np.float64)
    for i in range(1, P - 1):
        D2[i, i - 1] = 1.0
        D2[i, i] = -2.0
        D2[i, i + 1] = 1.0
    D2[0, 0:4] = [2.0, -5.0, 4.0, -1.0]
    D2[P - 1, P - 4:P] = [-1.0, 4.0, -5.0, 2.0]
    return D1, D2


def _weights_bf16():
    """[128, 14, 128] bf16 lhsT blocks (same layout as v2):
    0-3: D1 blocks; 4-7: 4*(D2-2I) blocks; 8: 4I; 9: I;
    10-13: 4*D2 blocks (edge pipeline).
    All entries are small integers -> exact in bf16."""
    import ml_dtypes
    D1, D2 = _stencils()
    WR2 = 4.0 * (D2 - 2.0 * np.eye(P))
    WR2E = 4.0 * D2
    wtb = np.zeros((128, 14, 128), dtype=np.float64)
    for m in range(2):
        for kb in range(2):
            blk = lambda W: W[m * 128:(m + 1) * 128, kb * 128:(kb + 1) * 128].T
            wtb[:, m * 2 + kb, :] = blk(D1)
            wtb[:, 4 + m * 2 + kb, :] = blk(WR2)
            wtb[:, 10 + m * 2 + kb, :] = blk(WR2E)
    wtb[:, 8, :] = 4.0 * np.eye(128)
    wtb[:, 9, :] = np.eye(128)
    return wtb.astype(ml_dtypes.bfloat16)


def _weights_fp8():
    """[128, 2, 2, 128] fp8e4 DoubleRow lhsT: block m holds the two
    k-plane D1 blocks (diag + cross) for output row-block m.
    D1 entries are small integers -> exact in e4m3."""
    import ml_dtypes
    D1, _ = _stencils()
    w = np.zeros((128, 2, 2, 128), dtype=np.float64)
    for m in range(2):
        for kb in range(2):
            w[:, m, kb, :] = D1[m * 128:(m + 1) * 128,
                                kb * 128:(kb + 1) * 128].T
    return w.astype(ml_dtypes.float8_e4m3)


def _build_program():
    from concourse import bacc
    import concourse.mybir as mybir
    from concourse.tile import TileContext

    f32 = mybir.dt.float32
    bf16 = mybir.dt.bfloat16
    f8 = mybir.dt.float8e4
    ADD = mybir.AluOpType.add
    SUB = mybir.AluOpType.subtract
    MUL = mybir.AluOpType.mult
    COPY = mybir.ActivationFunctionType.Copy
    DR = mybir.MatmulPerfMode.DoubleRow

    nc = bacc.Bacc("TRN2", target_bir_lowering=False, debug=False,
                   num_devices=NCORES)
    xe = nc.dram_tensor("xe", [128, 2, 2, BPC, 8], bf16, kind="ExternalInput")
    # ch 0 = X0' = x0+1.5, ch 1 = x1; one DMA per chunk for both
    xb = nc.dram_tensor("xb", [CHUNKS, 128, 2, 2, BCH, P], bf16,
                        kind="ExternalInput")
    xf = nc.dram_tensor("xf", [CHUNKS, 128, 2, 2, BCH, P], f8,
                        kind="ExternalInput")
    wtbd = nc.dram_tensor("wtbd", [128, 14, 128], bf16, kind="ExternalInput")
    wdrd = nc.dram_tensor("wdrd", [128, 2, 2, 128], f8, kind="ExternalInput")
    # X1 row 127 per chunk, relocated to partition 0 (PE rhs base must be
    # 0/32/64; row 127 lives at partition 127 in the main layout)
    xr1d = nc.dram_tensor("xr1d", [1, CHUNKS, BCH, P], bf16,
                          kind="ExternalInput")
    yout = nc.dram_tensor("yout", [CHUNKS, 128, 2, BCH, P], bf16,
                          kind="ExternalOutput")

    with TileContext(nc) as tc:
        with (
            tc.tile_pool(name="const", bufs=1) as cpool,
            tc.tile_pool(name="edge", bufs=1) as epool,
            tc.tile_pool(name="work", bufs=2) as pool,
            tc.tile_pool(name="psum", bufs=2, space="PSUM") as pp,
        ):
            wtb = cpool.tile([128, 14, 128], bf16)
            wdr = cpool.tile([128, 2, 2, 128], f8)
            xr1 = cpool.tile([1, CHUNKS, BCH, P], bf16)
            X0e = epool.tile([128, 2, BPC, 8], bf16)
            X1e = epool.tile([128, 2, BPC, 8], bf16)

            def emit_const_dmas():
                # issue order tuned for pipeline start: wdr feeds the first
                # DoubleRow matmuls, wtb blocks 4..9 feed chunk-0's u4 and
                # merge, the rest (edge blocks) can land later.  Everything
                # is split so no single queue carries a >130KB transfer.
                nc.sync.dma_start(out=wdr[:], in_=wdrd[:])
                # xe rides the GPSIMD (SWDGE) queue: idle until mid-kernel,
                # so these 8 issues are free and do not delay chunk inputs
                for k in range(2):
                    for bh in range(2):
                        bsl = slice(bh * (BPC // 2), (bh + 1) * (BPC // 2))
                        nc.gpsimd.dma_start(out=X0e[:, k, bsl],
                                            in_=xe[:, 0, k, bsl])
                        nc.gpsimd.dma_start(out=X1e[:, k, bsl],
                                            in_=xe[:, 1, k, bsl])
                for blks in (slice(4, 10), slice(0, 4), slice(10, 14)):
                    nc.sync.dma_start(out=wtb[:, blks], in_=wtbd[:, blks])
                nc.sync.dma_start(out=xr1[:], in_=xr1d[:])

            def Wb(i):
                return wtb[:, i, :]

            stt = nc.vector.scalar_tensor_tensor

            def emit_edge():
                # ------------- edge pipeline (output cols j=0 and j=255) ----
                X0ef = X0e.rearrange("p k b c -> p (k b c)")
                X1ef = X1e.rearrange("p k b c -> p (k b c)")
                # [128, 32, 8] views
                E1 = X1e.rearrange("p k b c -> p (k b) c")
                E0 = X0e.rearrange("p k b c -> p (k b) c")

                def et(name, d=2):
                    return epool.tile([128, 2 * BPC, d], f32, name=name,
                                      tag=name)

                esr = pp.tile([128, 3, 2, BPC, 8], f32, tag="sr", bufs=2)
                S1e, R1e, R2e = esr[:, 0], esr[:, 1], esr[:, 2]
                R2ef = R2e.rearrange("p k b c -> p (k b c)")
                R1ef = R1e.rearrange("p k b c -> p (k b c)")
                S1ef = S1e.rearrange("p k b c -> p (k b c)")
                # complete each accumulation group before starting the next
                # (interleaved start/stop groups sharing a PSUM bank corrupt
                # partials)
                for m in range(2):
                    osl = slice(m * 128, (m + 1) * 128)
                    for outf, wb0, rhs in (
                        (R1ef, 0, X1ef), (S1ef, 0, X0ef), (R2ef, 10, X1ef),
                    ):
                        for kb in range(2):
                            isl = slice(kb * 128, (kb + 1) * 128)
                            nc.tensor.matmul(outf[:, osl], Wb(wb0 + m * 2 + kb),
                                             rhs[:, isl], start=kb == 0,
                                             stop=kb == 1)

                # one ACT op evacuates all three stencil tensors to SBUF so
                # the whole elementwise chain can run on the otherwise-idle
                # GPSIMD engine, off the saturated DVE stream
                Ecp = epool.tile([128, 3, 2, BPC, 8], f32)
                nc.scalar.copy(
                    out=Ecp.rearrange("p t k b c -> p (t k b c)"),
                    in_=esr.rearrange("p t k b c -> p (t k b c)"))
                SP = Ecp[:, 0].rearrange("p k b c -> p (k b) c")
                RP1s = Ecp[:, 1].rearrange("p k b c -> p (k b) c")
                RP2s = Ecp[:, 2].rearrange("p k b c -> p (k b) c")

                # paired forward/mirrored diffs: half 0 = j=0 side (fwd),
                # half 1 = j=255 side (also forward-oriented: f7-f6 etc.)
                # TT-shaped ops ride GPSIMD; stt is not supported on Pool,
                # so stt-shaped ops stay on DVE (tiny)
                a1, b1, c1 = et("a1"), et("b1"), et("c1")
                a0, b0 = et("a0"), et("b0")
                nc.gpsimd.tensor_sub(a1[:], E1[:, :, 1:8:6], E1[:, :, 0:7:6])
                nc.gpsimd.tensor_sub(b1[:], E1[:, :, 2:7:4], E1[:, :, 1:6:4])
                nc.gpsimd.tensor_sub(c1[:], E1[:, :, 3:6:2], E1[:, :, 2:5:2])
                nc.gpsimd.tensor_sub(a0[:], E0[:, :, 1:8:6], E0[:, :, 0:7:6])
                nc.gpsimd.tensor_sub(b0[:], E0[:, :, 2:7:4], E0[:, :, 1:6:4])

                # one-sided raw stencils (Z sign flips on the mirror half)
                q, Z = et("q"), et("Z")
                C1pe, C1ae = et("C1pe"), et("C1ae")
                stt(q[:], b1[:], 3.0, c1[:], MUL, SUB)      # 3b - c
                stt(Z[:], a1[:], -2.0, q[:], MUL, ADD)      # -2a + 3b - c
                stt(C1pe[:], a1[:], 3.0, b1[:], MUL, SUB)   # 3a - b
                stt(C1ae[:], a0[:], 3.0, b0[:], MUL, SUB)

                U4e, tme, t2e = et("U4e"), et("tme"), et("t2e")
                stt(U4e[:, :, 0:1], Z[:, :, 0:1], 4.0, RP2s[:, :, 0:1],
                    MUL, ADD)
                stt(U4e[:, :, 1:2], Z[:, :, 1:2], -4.0, RP2s[:, :, 7:8],
                    MUL, ADD)

                stt(tme[:], E0[:, :, 0:8:7], 1.5, U4e[:], ADD, MUL)
                nc.gpsimd.tensor_mul(t2e[:], SP[:, :, 0:8:7],
                                     RP1s[:, :, 0:8:7])
                nc.gpsimd.tensor_add(tme[:], tme[:], t2e[:])
                nc.gpsimd.tensor_mul(C1ae[:], C1ae[:], C1pe[:])  # t3e
                nc.gpsimd.tensor_add(tme[:], tme[:], C1ae[:])
                rese = epool.tile([128, 2, BPC, 2], f32)
                nc.scalar.activation(
                    rese.rearrange("p k b e -> p (k b) e"), tme[:], COPY,
                    bias=-1.0, scale=-GAMMA)

                return rese

            # ------------- main pipeline, 8 chunks of 2 images -------------
            pending_tail = None
            for c in range(CHUNKS):
                # bf16 x1 into 2-left-padded tile (shifted 4I rhs views);
                # x0' plain; fp8 copies for the DoubleRow D1 matmuls
                # one tile for both bf16 channels; the two pad slots sit
                # between / after the planes so the +-1-shifted X1 views
                # exist (they read pad garbage only into never-read slots)
                XB = pool.tile([128, 2, FCH + 2], bf16, tag="xb", bufs=4)
                XBd = XB[:, :, 0:FCH].rearrange(
                    "p c (k b j) -> p c k b j", k=2, b=BCH)
                F = pool.tile([128, 2, 2, BCH * P], f8, tag="xf", bufs=4)
                if c < 2:
                    # split the first chunks' loads four ways and issue from
                    # both HWDGE queues (SP + ACT) so the pipeline starts
                    # fast: a single 256KB DMA takes ~13us on one queue
                    for k in range(2):
                        for ch in range(2):
                            eng = nc.scalar if (c == 0 and ch == 1) else nc.sync
                            eng.dma_start(out=XBd[:, ch, k],
                                          in_=xb[c, :, ch, k])
                        feng = nc.scalar if c == 0 else nc.sync
                        feng.dma_start(out=F[:, :, k], in_=xf[c, :, :, k])
                else:
                    nc.sync.dma_start(out=XBd, in_=xb[c])
                    nc.sync.dma_start(
                        out=F.rearrange("p t k (b j) -> p t k b j", b=BCH),
                        in_=xf[c])
                if c == 0:
                    emit_const_dmas()

                XBf = XB.rearrange("p c f -> p (c f)")
                XOFF = FCH + 2               # X1 flat base in XBf
                X0f = XBf[:, 0:FCH]
                X1f = XBf[:, XOFF:XOFF + FCH]

                C1p = pool.tile([128, FCH], bf16, tag="c1p", bufs=3)
                C1a = pool.tile([128, FCH], bf16, tag="c1a", bufs=3)
                t3b = pool.tile([128, FCH], bf16, tag="t3b", bufs=3)
                sm1 = pool.tile([128, FCH], bf16, tag="sm1", bufs=3)
                t2b = pool.tile([128, FCH], bf16, tag="t2b", bufs=3)

                tm = pool.tile([128, FCH], bf16, tag="tm", bufs=3)
                SRb = pool.tile([128, 2, 2, P * BCH - 1], bf16, tag="srb",
                                bufs=3)

                # column stencils for BOTH channels in one op: the two
                # planes of XB sit at the same relative offsets, so a
                # 2-level view computes C1a (ch0) and C1p (ch1) together
                nc.vector.tensor_sub(C1p[:, 0:FCH - 2],
                                     XBf[:, XOFF + 2:XOFF + FCH],
                                     XBf[:, XOFF:XOFF + FCH - 2])
                nc.vector.tensor_sub(C1a[:, 0:FCH - 2], X0f[:, 2:FCH],
                                     X0f[:, 0:FCH - 2])
                # t3 split: GPSIMD computes the first half (slow but
                # otherwise idle, gets a head start), DVE the second, so
                # sum1 never stalls on the GPSIMD result
                HF = FCH // 2
                nc.gpsimd.tensor_mul(t3b[:, 0:HF], C1a[:, 0:HF],
                                     C1p[:, 0:HF])
                nc.vector.tensor_mul(t3b[:, HF:FCH], C1a[:, HF:FCH],
                                     C1p[:, HF:FCH])

                # S1 | R1: one fp8 DoubleRow matmul each per m (diag+cross
                # k-planes fused).  sr tile [128, 2, 512]: plane 0 = S1,
                # plane 1 = R1 for row-block m.
                # u4 and res share one tag: u4(c) is dead once tm(c) reads
                # it, and the merge that writes res(c) already waits on
                # tm(c), so the rotation slots interleave cleanly.
                u4 = pp.tile([128, 2, BCH * P], f32, name=f"u4_{c}",
                             tag="uw", bufs=2)
                # both DoubleRow pairs and their ACT evacuations FIRST:
                # evac-m1 gates the t2 product, so it must not queue behind
                # u4's matmuls on the PE stream
                srs = []
                for m in range(2):
                    sr = pp.tile([128, 2, BCH * P], f32, name=f"sr_{c}_{m}",
                                 tag="sr", bufs=2)
                    srs.append(sr)
                    nc.tensor.matmul(sr[:, 0, :], wdr[:, m], F[:, 0],
                                     start=True, stop=True, perf_mode=DR)
                    nc.tensor.matmul(sr[:, 1, :], wdr[:, m], F[:, 1],
                                     start=True, stop=True, perf_mode=DR)
                    # ACT evacuates S1|R1 with the shift-1 baked in
                    nc.scalar.copy(out=SRb[:, m], in_=sr[:, :, 1:BCH * P])

                for m in range(2):
                    # U4 row part: WR2 diag block + contract-1 cross fixup
                    isl = slice(m * (BCH * P), (m + 1) * (BCH * P))
                    nc.tensor.matmul(u4[:, m, :], Wb(4 + m * 3), X1f[:, isl],
                                     start=True, stop=False)
                    if m == 0:
                        # out row 127 += 4 * X1[row 128]
                        nc.tensor.matmul(u4[:, 0, :], wtb[0:1, 5, :],
                                         X1f[0:1, BCH * P:FCH],
                                         start=False, stop=False)
                    else:
                        # out row 128 += 4 * X1[row 127]; lhsT 4*e0 is
                        # row 0 of the 4I block
                        nc.tensor.matmul(
                            u4[:, 1, :], wtb[0:1, 8, :],
                            xr1[:, c].rearrange("p b j -> p (b j)"),
                            start=False, stop=False)
                    # U4 col part: 4I on +-1-shifted rhs views
                    lo = m * (BCH * P)
                    hi = lo + BCH * P
                    nc.tensor.matmul(u4[:, m, :], Wb(8),
                                     XBf[:, XOFF + lo + 1:XOFF + hi + 1],
                                     start=False, stop=False)
                    nc.tensor.matmul(u4[:, m, :], Wb(8),
                                     XBf[:, XOFF + lo - 1:XOFF + hi - 1],
                                     start=False, stop=True)

                # emit the previous chunk's tail here: its merge matmuls land
                # AFTER this chunk's stencil matmuls on the PE stream, which
                # breaks the tm -> u4 -> merge -> tm cross-engine cycle
                if pending_tail is not None:
                    pending_tail()
                    pending_tail = None

                # tm = X0' * U4 per m-half, stt straight from PSUM (each
                # half can start as soon as its u4 accumulation group stops)
                H = FCH // 2
                for m in range(2):
                    lo = m * H
                    stt(tm[:, lo:lo + H - 1], X0f[:, lo + 1:lo + H], 1.0,
                        u4[:, m, 1:H], MUL, MUL)

                # t2 = S1*R1 (bf16 2x), sum1 = t2 + t3 (bf16 2x)
                t2v = t2b.rearrange("p (m u) -> p m u", m=2)[:, :, 0:BCH * P - 1]
                nc.vector.tensor_mul(t2v, SRb[:, :, 0, :], SRb[:, :, 1, :])
                nc.vector.tensor_add(sm1[:], t2b[:], t3b[:])

                if c == 0:
                    rese = emit_edge()

                def tail(c=c, tm=tm, sm1=sm1):
                    # final merge on PE: res = I@tm + I@sum1 per 512-half
                    res = pp.tile([128, 2, BCH, P], f32, name=f"res_{c}",
                                  tag="uw", bufs=2)
                    resf = res.rearrange("p k b j -> p (k b j)")
                    for h in range(2):
                        lo, hi = h * H, h * H + H - 1
                        nc.tensor.matmul(resf[:, lo:hi], Wb(9), tm[:, lo:hi],
                                         start=True, stop=False)
                        nc.tensor.matmul(resf[:, lo:hi], Wb(9),
                                         sm1[:, lo:hi],
                                         start=False, stop=True)
                    outt = pool.tile([128, 2, BCH, P], bf16, tag="out",
                                     bufs=3)
                    # res slot t = col t+1 -> out col j reads slot j-1
                    nc.scalar.activation(outt[:, :, :, 1:P - 1],
                                         res[:, :, :, 0:P - 2], COPY,
                                         bias=-1.0, scale=-GAMMA)
                    # edge columns j=0,255 from the edge pipeline (one copy)
                    nc.scalar.copy(out=outt[:, :, :, 0:P:P - 1],
                                   in_=rese[:, :, c * BCH:(c + 1) * BCH, :])
                    nc.gpsimd.dma_start(out=yout[c], in_=outt[:])

                pending_tail = tail
            pending_tail()

    nc.compile()
    return nc


def _get_program():
    if "nc" not in _cache:
        _cache["nc"] = _build_program()
        _cache["wtbd"] = _weights_bf16()
        _cache["wdrd"] = _weights_fp8()
    return _cache["nc"], _cache["wtbd"], _cache["wdrd"]


def _shard_inputs(x0_pred):
    import ml_dtypes
    x = np.ascontiguousarray(np.asarray(x0_pred, dtype=np.float32))
    _, wtbd, wdrd = _get_program()
    in_maps = []
    for i in range(NCORES):
        shard = x[i * BPC:(i + 1) * BPC]                      # [16,2,256,256]
        x0p = shard[:, 0] + 1.5                               # [16,256,256]
        x1 = shard[:, 1]
        # [chunks, 128, k, b, j] layout: img = 2c+b, row = 128k+p

        def to_chunks(a):
            # [16,256,256] -> [8,2,2,128,256] (c,b,k,p,j) -> (c,p,k,b,j)
            r = a.reshape(CHUNKS, BCH, 2, 128, P).transpose(0, 3, 2, 1, 4)
            return np.ascontiguousarray(r)

        c0 = to_chunks(x0p)
        c1 = to_chunks(x1)
        # [c, p, ch, k, b, j]
        xbm = np.ascontiguousarray(
            np.stack([c0, c1], axis=2)).astype(ml_dtypes.bfloat16)
        xfm = np.ascontiguousarray(
            np.stack([c0, c1], axis=2)).astype(ml_dtypes.float8_e4m3)
        # X1 row 127: [16,256] -> [1, 8c, 2b, 256]
        xr1 = np.ascontiguousarray(
            x1[:, 127, :].reshape(1, CHUNKS, BCH, P)).astype(
                ml_dtypes.bfloat16)

        # edge tensor [128, ch, k, b16, 8cols] from raw x0, x1 (the edge
        # pipeline applies the +1.5 itself)
        both = np.stack([shard[:, 0], x1], axis=1)            # [16,2,256,256]
        arr = both.reshape(BPC, 2, 2, 128, P).transpose(3, 1, 2, 0, 4)
        cols = [0, 1, 2, 3, P - 4, P - 3, P - 2, P - 1]
        xe = np.ascontiguousarray(arr[:, :, :, :, cols]).astype(
            ml_dtypes.bfloat16)
        in_maps.append({"xe": xe, "xb": xbm, "xf": xfm,
                        "xr1d": xr1, "wtbd": wtbd, "wdrd": wdrd})
    return in_maps


def _unshard(results):
    outs = []
    for i in range(NCORES):
        y = np.asarray(results[i]["yout"], dtype=np.float32)
        # [8, 128, 2, 2, 256] (c,p,k,b,j) -> img 2c+b, row 128k+p
        y = y.transpose(0, 3, 2, 1, 4).reshape(BPC, 1, P, P)
        outs.append(y)
    return np.ascontiguousarray(np.concatenate(outs, axis=0))


def _run(x0_pred, trace=False, tmpdir=None):
    import time
    from concourse.bass_utils import run_bass_kernel_spmd
    nc = _get_program()[0]
    in_maps = _shard_inputs(x0_pred)
    try:
        res = run_bass_kernel_spmd(nc, in_maps, list(range(NCORES)),
                                   trace=trace, tmpdir=tmpdir)
    except Exception:
        # transient NRT execution failures have been observed; one retry
        time.sleep(2.0)
        res = run_bass_kernel_spmd(nc, in_maps, list(range(NCORES)),
                                   trace=trace, tmpdir=tmpdir)
    return _unshard(res.results), res


def kernel(x0_pred):
    out, _ = _run(x0_pred, trace=False)
    return out


# revision 67
# speedup vs baseline: 1.1586x; 1.1586x over previous
"""Trainium2 Bass kernel for nn_DarcyResidual (P=256, B=128, 8 NeuronCores).

Math (reference):
    a = (x0 + 1.5) / 0.2,  p = (x1 + 0.9) / 115
    residual = -a*(p_d00 + p_d11) - a_d0*p_d0 - a_d1*p_d1 - 1
2nd-order central differences inside, 2nd-order one-sided at borders,
h = 1/256 on both axes.

Folded form computed here (G = 5/(460 h^2)):
    residual = -G * [ (X0 + 1.5)*U4 + S1*R1 + C1a*C1p ] - 1
      U4  = 4*(rowD2raw(X1) + colD2raw(X1))   (raw h^2-scaled 2nd diffs)
      R1  = rowD1raw(X1), S1 = rowD1raw(X0)   (raw 2h-scaled 1st diffs)
      C1p = colD1raw(X1), C1a = colD1raw(X0)

v2 layout per core (16 images): SBUF [partition = row-within-128-block,
free = (row-block k:2, image b, col j:256)], 8 chunks of 2 images.
All row-direction (d0) stencils are bf16 TensorE matmuls (banded stencil
matrices as lhsT blocks); the column Laplacian rides the same PSUM
accumulation via 4I matmuls on +-1-column-shifted rhs views of a padded
bf16 x1 tile.  S1*R1 is a DVE stt directly from PSUM (shifted bf16 out);
C1a*C1p comes from shifted-aligned 2x bf16 DVE stencils.  The three
terms are summed in a PSUM "res" bank by two identity bf16 matmuls
(rhs = tm and the shifted gradient sum), and ScalarE does the single
fused evacuate+affine(-G,-1)+bf16-cast.  Border columns j=0,255 keep
the f32r edge pipeline fed by a host-pregathered edge tensor.  Output
is bf16 (upcast on host); output DMAs ride the GPSIMD (SWDGE) queue.
"""

import numpy as np

P = 256
B = 128
NCORES = 8
BPC = B // NCORES          # images per core = 16
CHUNKS = 8
BCH = BPC // CHUNKS        # images per chunk = 2
FCH = 2 * BCH * P          # chunk free size = 1024
GAMMA = 5.0 * 65536.0 / 460.0

_cache = {}


def _stencils():
    D1 = np.zeros((P, P), dtype=np.float64)
    for i in range(1, P - 1):
        D1[i, i - 1] = -1.0
        D1[i, i + 1] = 1.0
    D1[0, 0:3] = [-3.0, 4.0, -1.0]
    D1[P - 1, P - 3:P] = [1.0, -4.0, 3.0]

    D2 = np.zeros((P, P), dtype=np.float64)
    for i in range(1, P - 1):
        D2[i, i - 1] = 1.0
        D2[i, i] = -2.0
        D2[i, i + 1] = 1.0
    D2[0, 0:4] = [2.0, -5.0, 4.0, -1.0]
    D2[P - 1, P - 4:P] = [-1.0, 4.0, -5.0, 2.0]
    return D1, D2


def _weights_bf16():
    """[128, 14, 128] bf16 lhsT blocks for all matmuls.
    0-3: D1 blocks; 4-7: 4*(D2-2I) blocks; 8: 4I; 9: I;
    10-13: 4*D2 blocks (edge pipeline, col stencil complete).
    All entries are small integers -> exact in bf16."""
    import ml_dtypes
    D1, D2 = _stencils()
    WR2 = 4.0 * (D2 - 2.0 * np.eye(P))
    WR2E = 4.0 * D2
    wtb = np.zeros((128, 14, 128), dtype=np.float64)
    for m in range(2):
        for kb in range(2):
            blk = lambda W: W[m * 128:(m + 1) * 128, kb * 128:(kb + 1) * 128].T
            wtb[:, m * 2 + kb, :] = blk(D1)
            wtb[:, 4 + m * 2 + kb, :] = blk(WR2)
            wtb[:, 10 + m * 2 + kb, :] = blk(WR2E)
    wtb[:, 8, :] = 4.0 * np.eye(128)
    wtb[:, 9, :] = np.eye(128)
    return wtb.astype(ml_dtypes.bfloat16)


def _build_program():
    from concourse import bacc
    import concourse.mybir as mybir
    from concourse.tile import TileContext

    f32 = mybir.dt.float32
    f32r = mybir.dt.float32r
    bf16 = mybir.dt.bfloat16
    ADD = mybir.AluOpType.add
    SUB = mybir.AluOpType.subtract
    MUL = mybir.AluOpType.mult
    COPY = mybir.ActivationFunctionType.Copy

    nc = bacc.Bacc("TRN2", target_bir_lowering=False, debug=False,
                   num_devices=NCORES)
    xe = nc.dram_tensor("xe", [128, 2, 2, BPC, 8], bf16, kind="ExternalInput")
    xb = nc.dram_tensor("xb", [128, 2, 2, BPC, P], bf16, kind="ExternalInput")
    wtbd = nc.dram_tensor("wtbd", [128, 14, 128], bf16, kind="ExternalInput")
    yout = nc.dram_tensor("yout", [128, 2, BPC, P], bf16, kind="ExternalOutput")

    with TileContext(nc) as tc:
        with (
            tc.tile_pool(name="const", bufs=1) as cpool,
            tc.tile_pool(name="edge", bufs=1) as epool,
            tc.tile_pool(name="work", bufs=2) as pool,
            tc.tile_pool(name="psum", bufs=2, space="PSUM") as pp,
        ):
            # chunk-0 bf16 inputs first, then the small weight tensors.
            # x1 goes into a 2-left-padded tile so the +-1-column-shifted
            # identity-matmul rhs views exist and the C1p stencil views
            # stay 4-byte aligned (2x mode).
            # chunk-0 inputs first (split by k over more DMA queues so the
            # pipeline starts fast), then the weights in three pieces
            wtb = cpool.tile([128, 14, 128], bf16)
            Xp0 = pool.tile([128, FCH + 4], bf16, tag="x1", bufs=3)
            X0c0 = pool.tile([128, 2, BCH, P], bf16, tag="x0", bufs=3)
            Xp0v = Xp0[:, 2:FCH + 2].rearrange(
                "p (k b j) -> p k b j", k=2, b=BCH)
            for k in range(2):
                nc.sync.dma_start(out=Xp0v[:, k], in_=xb[:, 1, k, 0:BCH, :])
                nc.sync.dma_start(out=X0c0[:, k], in_=xb[:, 0, k, 0:BCH, :])
            for blks in (slice(0, 5), slice(5, 10), slice(10, 14)):
                nc.sync.dma_start(out=wtb[:, blks], in_=wtbd[:, blks])

            def Wb(i):
                return wtb[:, i, :]

            stt = nc.vector.scalar_tensor_tensor

            def emit_edge():
                # ------------- edge pipeline (output cols j=0 and j=255) -------
                X0e = epool.tile([128, 2, BPC, 8], bf16)
                X1e = epool.tile([128, 2, BPC, 8], bf16)
                nc.sync.dma_start(out=X0e[:], in_=xe[:, 0])
                nc.sync.dma_start(out=X1e[:], in_=xe[:, 1])

                X0ef = X0e.rearrange("p k b c -> p (k b c)")
                X1ef = X1e.rearrange("p k b c -> p (k b c)")
                # [128, 32, 8] views
                E1 = X1e.rearrange("p k b c -> p (k b) c")
                E0 = X0e.rearrange("p k b c -> p (k b) c")

                def et(name, d=2):
                    return epool.tile([128, 2 * BPC, d], f32, name=name, tag=name)

                if True:
                    R2e = pp.tile([128, 2, BPC, 8], f32, tag="r2")
                    R1e = pp.tile([128, 2, BPC, 8], f32, tag="r1")
                    S1e = pp.tile([128, 2, BPC, 8], f32, tag="s1")
                    R2ef = R2e.rearrange("p k b c -> p (k b c)")
                    R1ef = R1e.rearrange("p k b c -> p (k b c)")
                    S1ef = S1e.rearrange("p k b c -> p (k b c)")
                    for m in range(2):
                        osl = slice(m * 128, (m + 1) * 128)
                        for kb in range(2):
                            isl = slice(kb * 128, (kb + 1) * 128)
                            st, sp = kb == 0, kb == 1
                            nc.tensor.matmul(R1ef[:, osl], Wb(m * 2 + kb),
                                             X1ef[:, isl], start=st, stop=sp)
                            nc.tensor.matmul(S1ef[:, osl], Wb(m * 2 + kb),
                                             X0ef[:, isl], start=st, stop=sp)
                            nc.tensor.matmul(R2ef[:, osl], Wb(10 + m * 2 + kb),
                                             X1ef[:, isl], start=st, stop=sp)

                    # paired forward/mirrored diffs: half 0 = j=0 side (fwd),
                    # half 1 = j=255 side (also forward-oriented: f7-f6 etc.)
                    a1, b1, c1 = et("a1"), et("b1"), et("c1")
                    a0, b0 = et("a0"), et("b0")
                    nc.vector.tensor_sub(a1[:], E1[:, :, 1:8:6], E1[:, :, 0:7:6])
                    nc.vector.tensor_sub(b1[:], E1[:, :, 2:7:4], E1[:, :, 1:6:4])
                    nc.vector.tensor_sub(c1[:], E1[:, :, 3:6:2], E1[:, :, 2:5:2])
                    nc.vector.tensor_sub(a0[:], E0[:, :, 1:8:6], E0[:, :, 0:7:6])
                    nc.vector.tensor_sub(b0[:], E0[:, :, 2:7:4], E0[:, :, 1:6:4])

                    # one-sided raw stencils (Z sign flips on the mirror half)
                    q, Z = et("q"), et("Z")
                    C1pe, C1ae = et("C1pe"), et("C1ae")
                    stt(q[:], b1[:], 3.0, c1[:], MUL, SUB)      # 3b - c
                    stt(Z[:], a1[:], -2.0, q[:], MUL, ADD)      # -2a + 3b - c
                    stt(C1pe[:], a1[:], 3.0, b1[:], MUL, SUB)   # 3a - b
                    stt(C1ae[:], a0[:], 3.0, b0[:], MUL, SUB)

                    RP2 = R2e.rearrange("p k b c -> p (k b) c")
                    RP1 = R1e.rearrange("p k b c -> p (k b) c")
                    U4e, tme, t2e = et("U4e"), et("tme"), et("t2e")
                    stt(U4e[:, :, 0:1], Z[:, :, 0:1], 4.0, RP2[:, :, 0:1], MUL, ADD)
                    stt(U4e[:, :, 1:2], Z[:, :, 1:2], -4.0, RP2[:, :, 7:8], MUL, ADD)

                    Scpe = epool.tile([128, 2, BPC, 8], f32)
                    nc.scalar.copy(out=Scpe.rearrange("p k b c -> p (k b c)"),
                                   in_=S1ef[:])
                    SP = Scpe.rearrange("p k b c -> p (k b) c")

                    stt(tme[:], E0[:, :, 0:8:7], 1.5, U4e[:], ADD, MUL)
                    nc.vector.tensor_mul(t2e[:], SP[:, :, 0:8:7], RP1[:, :, 0:8:7])
                    nc.vector.tensor_add(tme[:], tme[:], t2e[:])
                    nc.vector.tensor_mul(C1ae[:], C1ae[:], C1pe[:])  # t3e in-place
                    nc.vector.tensor_add(tme[:], tme[:], C1ae[:])
                    rese = epool.tile([128, 2, BPC, 2], f32)
                    nc.scalar.activation(
                        rese.rearrange("p k b e -> p (k b) e"), tme[:], COPY,
                        bias=-1.0, scale=-GAMMA)

                return rese

            # ------------- main pipeline, 8 chunks of 2 images -------------
            if True:
                for c in range(CHUNKS):
                    b0c = c * BCH
                    if c == 0:
                        X0c, Xp = X0c0, Xp0
                    else:
                        X0c = pool.tile([128, 2, BCH, P], bf16, tag="x0",
                                        bufs=3)
                        Xp = pool.tile([128, FCH + 4], bf16, tag="x1",
                                       bufs=3)
                        Xpv = Xp[:, 2:FCH + 2].rearrange(
                            "p (k b j) -> p k b j", k=2, b=BCH)
                        if c == 1:
                            # chunk 1 also split by k for the fast ramp
                            for k in range(2):
                                nc.sync.dma_start(
                                    out=Xpv[:, k],
                                    in_=xb[:, 1, k, b0c:b0c + BCH, :])
                                nc.sync.dma_start(
                                    out=X0c[:, k],
                                    in_=xb[:, 0, k, b0c:b0c + BCH, :])
                        else:
                            nc.sync.dma_start(
                                out=Xpv, in_=xb[:, 1, :, b0c:b0c + BCH, :])
                            nc.sync.dma_start(
                                out=X0c[:],
                                in_=xb[:, 0, :, b0c:b0c + BCH, :])
                    X0f = X0c.rearrange("p k b j -> p (k b j)")
                    X1f = Xp[:, 2:FCH + 2]
                    C1p = pool.tile([128, FCH], bf16, tag="c1p", bufs=3)
                    C1a = pool.tile([128, FCH], bf16, tag="c1a", bufs=3)
                    t3b = pool.tile([128, FCH], bf16, tag="t3b", bufs=3)
                    t2b = pool.tile([128, FCH], bf16, tag="t2b", bufs=3)
                    rcp = pool.tile([128, FCH], bf16, tag="rcp", bufs=3)
                    u4b = pool.tile([128, FCH], bf16, tag="u4b", bufs=3)
                    tm = pool.tile([128, 2, BCH, P], bf16, tag="tm", bufs=3)
                    tmf = tm.rearrange("p k b j -> p (k b j)")

                    # column stencils, shifted layout (slot t = col t+1),
                    # all views 4-byte aligned -> 2x mode
                    nc.vector.tensor_sub(C1p[:, 0:FCH - 2], Xp[:, 4:FCH + 2],
                                         Xp[:, 2:FCH])
                    nc.vector.tensor_sub(C1a[:, 0:FCH - 2], X0f[:, 2:FCH],
                                         X0f[:, 0:FCH - 2])
                    nc.vector.tensor_mul(t3b[:], C1a[:], C1p[:])

                    res = pp.tile([128, 2, BCH, P], f32, name=f"res_{c}",
                                  tag="res", bufs=1)
                    resf = res.rearrange("p k b j -> p (k b j)")
                    for m in range(2):
                        R1s = pp.tile([128, 2 * P], f32, name=f"r1_{c}_{m}",
                                      tag="r1")
                        S1s = pp.tile([128, 2 * P], f32, name=f"s1_{c}_{m}",
                                      tag="s1")
                        U4s = pp.tile([128, 2 * P], f32, name=f"r2_{c}_{m}",
                                      tag="r2")
                        for kb in range(2):
                            st, sp = kb == 0, kb == 1
                            isl = slice(kb * (BCH * P), (kb + 1) * (BCH * P))
                            nc.tensor.matmul(R1s[:], Wb(m * 2 + kb),
                                             X1f[:, isl], start=st, stop=sp)
                            nc.tensor.matmul(S1s[:], Wb(m * 2 + kb),
                                             X0f[:, isl], start=st, stop=sp)
                            nc.tensor.matmul(U4s[:], Wb(4 + m * 2 + kb),
                                             X1f[:, isl], start=st, stop=False)
                        # column-neighbor sums via 4I with +-1-shifted rhs:
                        # U4 = W_R2@X1 + 4I@X1[+1] + 4I@X1[-1], all in PSUM
                        lo = m * (BCH * P)
                        hi = lo + 2 * P
                        nc.tensor.matmul(U4s[:], Wb(8),
                                         Xp[:, lo + 3:hi + 3],
                                         start=False, stop=False)
                        nc.tensor.matmul(U4s[:], Wb(8),
                                         Xp[:, lo + 1:hi + 1],
                                         start=False, stop=True)
                        # DVE can read only one PSUM operand per op: ScalarE
                        # evacuates R1 (shifted bf16), S1 stays in PSUM.
                        nc.scalar.copy(out=rcp[:, lo:hi - 1],
                                       in_=R1s[:, 1:2 * P])
                        # t2b slot t = S1*R1 at col t+1 (shifted bf16 out)
                        stt(t2b[:, lo:hi - 1], S1s[:, 1:2 * P], 1.0,
                            rcp[:, lo:hi - 1], MUL, MUL)
                        # ScalarE evacuates U4 so the tm stt runs all-bf16
                        # SBUF 4B-aligned -> DVE 2x mode
                        nc.scalar.copy(out=u4b[:, lo:hi], in_=U4s[:])
                        # tm = (X0 + 1.5) * U4  (bf16 out)
                        stt(tmf[:, lo:hi], X0f[:, lo:hi], 1.5, u4b[:, lo:hi],
                            ADD, MUL)

                    # gradient-product sum in shifted bf16 (2x), then the
                    # whole merge happens on TensorE in the res PSUM bank
                    nc.vector.tensor_add(t2b[:], t2b[:], t3b[:])
                    # PSUM-bank-sized (N<=512) merge matmuls
                    H = FCH // 2
                    nc.tensor.matmul(resf[:, 1:H], Wb(9), tmf[:, 1:H],
                                     start=True, stop=False)
                    nc.tensor.matmul(resf[:, 1:H], Wb(9), t2b[:, 0:H - 1],
                                     start=False, stop=True)
                    nc.tensor.matmul(resf[:, H:FCH], Wb(9), tmf[:, H:FCH],
                                     start=True, stop=False)
                    nc.tensor.matmul(resf[:, H:FCH], Wb(9),
                                     t2b[:, H - 1:FCH - 1],
                                     start=False, stop=True)

                    if c == 0:
                        rese = emit_edge()
                    outt = pool.tile([128, 2, BCH, P], bf16, tag="out", bufs=3)
                    nc.scalar.activation(outt[:, :, :, 1:P - 1],
                                         res[:, :, :, 1:P - 1], COPY,
                                         bias=-1.0, scale=-GAMMA)
                    # edge columns j=0,255 from the edge pipeline (one copy)
                    nc.scalar.copy(out=outt[:, :, :, 0:P:P - 1],
                                   in_=rese[:, :, b0c:b0c + BCH, :])
                    nc.gpsimd.dma_start(
                        out=yout[:, :, b0c:b0c + BCH, :], in_=outt[:])

    nc.compile()
    return nc


def _get_program():
    if "nc" not in _cache:
        _cache["nc"] = _build_program()
        _cache["wtbd"] = _weights_bf16()
    return _cache["nc"], _cache["wtbd"]


def _shard_inputs(x0_pred):
    import ml_dtypes
    x = np.ascontiguousarray(np.asarray(x0_pred, dtype=np.float32))
    _, wtbd = _get_program()
    in_maps = []
    for i in range(NCORES):
        shard = x[i * BPC:(i + 1) * BPC]                      # [16,2,256,256]
        arr = shard.reshape(BPC, 2, 2, 128, P).transpose(3, 1, 2, 0, 4)
        xbi = np.ascontiguousarray(arr).astype(ml_dtypes.bfloat16)
        cols = [0, 1, 2, 3, P - 4, P - 3, P - 2, P - 1]
        xe = np.ascontiguousarray(xbi[:, :, :, :, cols])
        in_maps.append({"xe": xe, "xb": xbi, "wtbd": wtbd})
    return in_maps


def _unshard(results):
    outs = []
    for i in range(NCORES):
        y = np.asarray(results[i]["yout"], dtype=np.float32)  # [128,2,16,256]
        outs.append(y.transpose(2, 1, 0, 3).reshape(BPC, 1, P, P))
    return np.ascontiguousarray(np.concatenate(outs, axis=0))


def _run(x0_pred, trace=False, tmpdir=None):
    import time
    from concourse.bass_utils import run_bass_kernel_spmd
    nc = _get_program()[0]
    in_maps = _shard_inputs(x0_pred)
    try:
        res = run_bass_kernel_spmd(nc, in_maps, list(range(NCORES)),
                                   trace=trace, tmpdir=tmpdir)
    except Exception:
        # transient NRT execution failures have been observed; one retry
        time.sleep(2.0)
        res = run_bass_kernel_spmd(nc, in_maps, list(range(NCORES)),
                                   trace=trace, tmpdir=tmpdir)
    return _unshard(res.results), res


def kernel(x0_pred):
    out, _ = _run(x0_pred, trace=False)
    return out



# revision 68
# speedup vs baseline: 1.1702x; 1.0101x over previous
"""Trainium2 Bass kernel for nn_DarcyResidual (P=256, B=128, 8 NeuronCores).

Math (reference):
    a = (x0 + 1.5) / 0.2,  p = (x1 + 0.9) / 115
    residual = -a*(p_d00 + p_d11) - a_d0*p_d0 - a_d1*p_d1 - 1
2nd-order central differences inside, 2nd-order one-sided at borders,
h = 1/256 on both axes.

Folded form computed here (G = 5/(460 h^2)):
    residual = -G * [ (X0 + 1.5)*U4 + S1*R1 + C1a*C1p ] - 1
      U4  = 4*(rowD2raw(X1) + colD2raw(X1))   (raw h^2-scaled 2nd diffs)
      R1  = rowD1raw(X1), S1 = rowD1raw(X0)   (raw 2h-scaled 1st diffs)
      C1p = colD1raw(X1), C1a = colD1raw(X0)

v2 layout per core (16 images): SBUF [partition = row-within-128-block,
free = (row-block k:2, image b, col j:256)], 8 chunks of 2 images.
All row-direction (d0) stencils are bf16 TensorE matmuls (banded stencil
matrices as lhsT blocks); the column Laplacian rides the same PSUM
accumulation via 4I matmuls on +-1-column-shifted rhs views of a padded
bf16 x1 tile.  S1*R1 is a DVE stt directly from PSUM (shifted bf16 out);
C1a*C1p comes from shifted-aligned 2x bf16 DVE stencils.  The three
terms are summed in a PSUM "res" bank by two identity bf16 matmuls
(rhs = tm and the shifted gradient sum), and ScalarE does the single
fused evacuate+affine(-G,-1)+bf16-cast.  Border columns j=0,255 keep
the f32r edge pipeline fed by a host-pregathered edge tensor.  Output
is bf16 (upcast on host); output DMAs ride the GPSIMD (SWDGE) queue.
"""

import numpy as np

P = 256
B = 128
NCORES = 8
BPC = B // NCORES          # images per core = 16
CHUNKS = 8
BCH = BPC // CHUNKS        # images per chunk = 2
FCH = 2 * BCH * P          # chunk free size = 1024
GAMMA = 5.0 * 65536.0 / 460.0

_cache = {}


def _stencils():
    D1 = np.zeros((P, P), dtype=np.float64)
    for i in range(1, P - 1):
        D1[i, i - 1] = -1.0
        D1[i, i + 1] = 1.0
    D1[0, 0:3] = [-3.0, 4.0, -1.0]
    D1[P - 1, P - 3:P] = [1.0, -4.0, 3.0]

    D2 = np.zeros((P, P), dtype=np.float64)
    for i in range(1, P - 1):
        D2[i, i - 1] = 1.0
        D2[i, i] = -2.0
        D2[i, i + 1] = 1.0
    D2[0, 0:4] = [2.0, -5.0, 4.0, -1.0]
    D2[P - 1, P - 4:P] = [-1.0, 4.0, -5.0, 2.0]
    return D1, D2


def _weights_bf16():
    """[128, 14, 128] bf16 lhsT blocks for all matmuls.
    0-3: D1 blocks; 4-7: 4*(D2-2I) blocks; 8: 4I; 9: I;
    10-13: 4*D2 blocks (edge pipeline, col stencil complete).
    All entries are small integers -> exact in bf16."""
    import ml_dtypes
    D1, D2 = _stencils()
    WR2 = 4.0 * (D2 - 2.0 * np.eye(P))
    WR2E = 4.0 * D2
    wtb = np.zeros((128, 14, 128), dtype=np.float64)
    for m in range(2):
        for kb in range(2):
            blk = lambda W: W[m * 128:(m + 1) * 128, kb * 128:(kb + 1) * 128].T
            wtb[:, m * 2 + kb, :] = blk(D1)
            wtb[:, 4 + m * 2 + kb, :] = blk(WR2)
            wtb[:, 10 + m * 2 + kb, :] = blk(WR2E)
    wtb[:, 8, :] = 4.0 * np.eye(128)
    wtb[:, 9, :] = np.eye(128)
    return wtb.astype(ml_dtypes.bfloat16)


def _build_program():
    from concourse import bacc
    import concourse.mybir as mybir
    from concourse.tile import TileContext

    f32 = mybir.dt.float32
    f32r = mybir.dt.float32r
    bf16 = mybir.dt.bfloat16
    ADD = mybir.AluOpType.add
    SUB = mybir.AluOpType.subtract
    MUL = mybir.AluOpType.mult
    COPY = mybir.ActivationFunctionType.Copy

    nc = bacc.Bacc("TRN2", target_bir_lowering=False, debug=False,
                   num_devices=NCORES)
    xe = nc.dram_tensor("xe", [128, 2, 2, BPC, 8], bf16, kind="ExternalInput")
    xb = nc.dram_tensor("xb", [128, 2, 2, BPC, P], bf16, kind="ExternalInput")
    wtbd = nc.dram_tensor("wtbd", [128, 14, 128], bf16, kind="ExternalInput")
    yout = nc.dram_tensor("yout", [128, 2, BPC, P], bf16, kind="ExternalOutput")

    with TileContext(nc) as tc:
        with (
            tc.tile_pool(name="const", bufs=1) as cpool,
            tc.tile_pool(name="edge", bufs=1) as epool,
            tc.tile_pool(name="work", bufs=2) as pool,
            tc.tile_pool(name="psum", bufs=2, space="PSUM") as pp,
        ):
            # chunk-0 bf16 inputs first, then the small weight tensors.
            # x1 goes into a 2-left-padded tile so the +-1-column-shifted
            # identity-matmul rhs views exist and the C1p stencil views
            # stay 4-byte aligned (2x mode).
            wtb = cpool.tile([128, 14, 128], bf16)
            nc.sync.dma_start(out=wtb[:], in_=wtbd[:])
            Xp0 = pool.tile([128, FCH + 4], bf16, tag="x1", bufs=3)
            nc.sync.dma_start(
                out=Xp0[:, 2:FCH + 2].rearrange(
                    "p (k b j) -> p k b j", k=2, b=BCH),
                in_=xb[:, 1, :, 0:BCH, :])
            X0c0 = pool.tile([128, 2, BCH, P], bf16, tag="x0", bufs=3)
            nc.sync.dma_start(out=X0c0[:], in_=xb[:, 0, :, 0:BCH, :])

            def Wb(i):
                return wtb[:, i, :]

            stt = nc.vector.scalar_tensor_tensor

            def emit_edge():
                # ------------- edge pipeline (output cols j=0 and j=255) -------
                X0e = epool.tile([128, 2, BPC, 8], bf16)
                X1e = epool.tile([128, 2, BPC, 8], bf16)
                nc.sync.dma_start(out=X0e[:], in_=xe[:, 0])
                nc.sync.dma_start(out=X1e[:], in_=xe[:, 1])

                X0ef = X0e.rearrange("p k b c -> p (k b c)")
                X1ef = X1e.rearrange("p k b c -> p (k b c)")
                # [128, 32, 8] views
                E1 = X1e.rearrange("p k b c -> p (k b) c")
                E0 = X0e.rearrange("p k b c -> p (k b) c")

                def et(name, d=2):
                    return epool.tile([128, 2 * BPC, d], f32, name=name, tag=name)

                if True:
                    R2e = pp.tile([128, 2, BPC, 8], f32, tag="r2")
                    R1e = pp.tile([128, 2, BPC, 8], f32, tag="r1")
                    S1e = pp.tile([128, 2, BPC, 8], f32, tag="s1")
                    R2ef = R2e.rearrange("p k b c -> p (k b c)")
                    R1ef = R1e.rearrange("p k b c -> p (k b c)")
                    S1ef = S1e.rearrange("p k b c -> p (k b c)")
                    for m in range(2):
                        osl = slice(m * 128, (m + 1) * 128)
                        for kb in range(2):
                            isl = slice(kb * 128, (kb + 1) * 128)
                            st, sp = kb == 0, kb == 1
                            nc.tensor.matmul(R1ef[:, osl], Wb(m * 2 + kb),
                                             X1ef[:, isl], start=st, stop=sp)
                            nc.tensor.matmul(S1ef[:, osl], Wb(m * 2 + kb),
                                             X0ef[:, isl], start=st, stop=sp)
                            nc.tensor.matmul(R2ef[:, osl], Wb(10 + m * 2 + kb),
                                             X1ef[:, isl], start=st, stop=sp)

                    # paired forward/mirrored diffs: half 0 = j=0 side (fwd),
                    # half 1 = j=255 side (also forward-oriented: f7-f6 etc.)
                    a1, b1, c1 = et("a1"), et("b1"), et("c1")
                    a0, b0 = et("a0"), et("b0")
                    nc.vector.tensor_sub(a1[:], E1[:, :, 1:8:6], E1[:, :, 0:7:6])
                    nc.vector.tensor_sub(b1[:], E1[:, :, 2:7:4], E1[:, :, 1:6:4])
                    nc.vector.tensor_sub(c1[:], E1[:, :, 3:6:2], E1[:, :, 2:5:2])
                    nc.vector.tensor_sub(a0[:], E0[:, :, 1:8:6], E0[:, :, 0:7:6])
                    nc.vector.tensor_sub(b0[:], E0[:, :, 2:7:4], E0[:, :, 1:6:4])

                    # one-sided raw stencils (Z sign flips on the mirror half)
                    q, Z = et("q"), et("Z")
                    C1pe, C1ae = et("C1pe"), et("C1ae")
                    stt(q[:], b1[:], 3.0, c1[:], MUL, SUB)      # 3b - c
                    stt(Z[:], a1[:], -2.0, q[:], MUL, ADD)      # -2a + 3b - c
                    stt(C1pe[:], a1[:], 3.0, b1[:], MUL, SUB)   # 3a - b
                    stt(C1ae[:], a0[:], 3.0, b0[:], MUL, SUB)

                    RP2 = R2e.rearrange("p k b c -> p (k b) c")
                    RP1 = R1e.rearrange("p k b c -> p (k b) c")
                    U4e, tme, t2e = et("U4e"), et("tme"), et("t2e")
                    stt(U4e[:, :, 0:1], Z[:, :, 0:1], 4.0, RP2[:, :, 0:1], MUL, ADD)
                    stt(U4e[:, :, 1:2], Z[:, :, 1:2], -4.0, RP2[:, :, 7:8], MUL, ADD)

                    Scpe = epool.tile([128, 2, BPC, 8], f32)
                    nc.scalar.copy(out=Scpe.rearrange("p k b c -> p (k b c)"),
                                   in_=S1ef[:])
                    SP = Scpe.rearrange("p k b c -> p (k b) c")

                    stt(tme[:], E0[:, :, 0:8:7], 1.5, U4e[:], ADD, MUL)
                    nc.vector.tensor_mul(t2e[:], SP[:, :, 0:8:7], RP1[:, :, 0:8:7])
                    nc.vector.tensor_add(tme[:], tme[:], t2e[:])
                    nc.vector.tensor_mul(C1ae[:], C1ae[:], C1pe[:])  # t3e in-place
                    nc.vector.tensor_add(tme[:], tme[:], C1ae[:])
                    rese = epool.tile([128, 2, BPC, 2], f32)
                    nc.scalar.activation(
                        rese.rearrange("p k b e -> p (k b) e"), tme[:], COPY,
                        bias=-1.0, scale=-GAMMA)

                return rese

            # ------------- main pipeline, 8 chunks of 2 images -------------
            if True:
                for c in range(CHUNKS):
                    b0c = c * BCH
                    if c == 0:
                        X0c, Xp = X0c0, Xp0
                    else:
                        X0c = pool.tile([128, 2, BCH, P], bf16, tag="x0",
                                        bufs=3)
                        Xp = pool.tile([128, FCH + 4], bf16, tag="x1",
                                       bufs=3)
                        nc.sync.dma_start(
                            out=Xp[:, 2:FCH + 2].rearrange(
                                "p (k b j) -> p k b j", k=2, b=BCH),
                            in_=xb[:, 1, :, b0c:b0c + BCH, :])
                        nc.sync.dma_start(out=X0c[:],
                                          in_=xb[:, 0, :, b0c:b0c + BCH, :])
                    X0f = X0c.rearrange("p k b j -> p (k b j)")
                    X1f = Xp[:, 2:FCH + 2]
                    C1p = pool.tile([128, FCH], bf16, tag="c1p", bufs=3)
                    C1a = pool.tile([128, FCH], bf16, tag="c1a", bufs=3)
                    t3b = pool.tile([128, FCH], bf16, tag="t3b", bufs=3)
                    t2b = pool.tile([128, FCH], bf16, tag="t2b", bufs=3)
                    rcp = pool.tile([128, FCH], bf16, tag="rcp", bufs=3)
                    u4b = pool.tile([128, FCH], bf16, tag="u4b", bufs=3)
                    tm = pool.tile([128, 2, BCH, P], bf16, tag="tm", bufs=3)
                    tmf = tm.rearrange("p k b j -> p (k b j)")

                    # column stencils, shifted layout (slot t = col t+1),
                    # all views 4-byte aligned -> 2x mode
                    nc.vector.tensor_sub(C1p[:, 0:FCH - 2], Xp[:, 4:FCH + 2],
                                         Xp[:, 2:FCH])
                    nc.vector.tensor_sub(C1a[:, 0:FCH - 2], X0f[:, 2:FCH],
                                         X0f[:, 0:FCH - 2])
                    nc.vector.tensor_mul(t3b[:], C1a[:], C1p[:])

                    res = pp.tile([128, 2, BCH, P], f32, name=f"res_{c}",
                                  tag="res", bufs=1)
                    resf = res.rearrange("p k b j -> p (k b j)")
                    for m in range(2):
                        R1s = pp.tile([128, 2 * P], f32, name=f"r1_{c}_{m}",
                                      tag="r1")
                        S1s = pp.tile([128, 2 * P], f32, name=f"s1_{c}_{m}",
                                      tag="s1")
                        U4s = pp.tile([128, 2 * P], f32, name=f"r2_{c}_{m}",
                                      tag="r2")
                        for kb in range(2):
                            st, sp = kb == 0, kb == 1
                            isl = slice(kb * (BCH * P), (kb + 1) * (BCH * P))
                            nc.tensor.matmul(R1s[:], Wb(m * 2 + kb),
                                             X1f[:, isl], start=st, stop=sp)
                            nc.tensor.matmul(S1s[:], Wb(m * 2 + kb),
                                             X0f[:, isl], start=st, stop=sp)
                            nc.tensor.matmul(U4s[:], Wb(4 + m * 2 + kb),
                                             X1f[:, isl], start=st, stop=False)
                        # column-neighbor sums via 4I with +-1-shifted rhs:
                        # U4 = W_R2@X1 + 4I@X1[+1] + 4I@X1[-1], all in PSUM
                        lo = m * (BCH * P)
                        hi = lo + 2 * P
                        nc.tensor.matmul(U4s[:], Wb(8),
                                         Xp[:, lo + 3:hi + 3],
                                         start=False, stop=False)
                        nc.tensor.matmul(U4s[:], Wb(8),
                                         Xp[:, lo + 1:hi + 1],
                                         start=False, stop=True)
                        # DVE can read only one PSUM operand per op: ScalarE
                        # evacuates R1 (shifted bf16), S1 stays in PSUM.
                        nc.scalar.copy(out=rcp[:, lo:hi - 1],
                                       in_=R1s[:, 1:2 * P])
                        # t2b slot t = S1*R1 at col t+1 (shifted bf16 out)
                        stt(t2b[:, lo:hi - 1], S1s[:, 1:2 * P], 1.0,
                            rcp[:, lo:hi - 1], MUL, MUL)
                        # ScalarE evacuates U4 so the tm stt runs all-bf16
                        # SBUF 4B-aligned -> DVE 2x mode
                        nc.scalar.copy(out=u4b[:, lo:hi], in_=U4s[:])
                        # tm = (X0 + 1.5) * U4  (bf16 out)
                        stt(tmf[:, lo:hi], X0f[:, lo:hi], 1.5, u4b[:, lo:hi],
                            ADD, MUL)

                    # gradient-product sum in shifted bf16 (2x), then the
                    # whole merge happens on TensorE in the res PSUM bank
                    nc.vector.tensor_add(t2b[:], t2b[:], t3b[:])
                    # PSUM-bank-sized (N<=512) merge matmuls
                    H = FCH // 2
                    nc.tensor.matmul(resf[:, 1:H], Wb(9), tmf[:, 1:H],
                                     start=True, stop=False)
                    nc.tensor.matmul(resf[:, 1:H], Wb(9), t2b[:, 0:H - 1],
                                     start=False, stop=True)
                    nc.tensor.matmul(resf[:, H:FCH], Wb(9), tmf[:, H:FCH],
                                     start=True, stop=False)
                    nc.tensor.matmul(resf[:, H:FCH], Wb(9),
                                     t2b[:, H - 1:FCH - 1],
                                     start=False, stop=True)

                    if c == 0:
                        rese = emit_edge()
                    outt = pool.tile([128, 2, BCH, P], bf16, tag="out", bufs=3)
                    nc.scalar.activation(outt[:, :, :, 1:P - 1],
                                         res[:, :, :, 1:P - 1], COPY,
                                         bias=-1.0, scale=-GAMMA)
                    # edge columns j=0,255 from the edge pipeline (one copy)
                    nc.scalar.copy(out=outt[:, :, :, 0:P:P - 1],
                                   in_=rese[:, :, b0c:b0c + BCH, :])
                    nc.gpsimd.dma_start(
                        out=yout[:, :, b0c:b0c + BCH, :], in_=outt[:])

    nc.compile()
    return nc


def _get_program():
    if "nc" not in _cache:
        _cache["nc"] = _build_program()
        _cache["wtbd"] = _weights_bf16()
    return _cache["nc"], _cache["wtbd"]


def _shard_inputs(x0_pred):
    import ml_dtypes
    x = np.ascontiguousarray(np.asarray(x0_pred, dtype=np.float32))
    _, wtbd = _get_program()
    in_maps = []
    for i in range(NCORES):
        shard = x[i * BPC:(i + 1) * BPC]                      # [16,2,256,256]
        arr = shard.reshape(BPC, 2, 2, 128, P).transpose(3, 1, 2, 0, 4)
        xbi = np.ascontiguousarray(arr).astype(ml_dtypes.bfloat16)
        cols = [0, 1, 2, 3, P - 4, P - 3, P - 2, P - 1]
        xe = np.ascontiguousarray(xbi[:, :, :, :, cols])
        in_maps.append({"xe": xe, "xb": xbi, "wtbd": wtbd})
    return in_maps


def _unshard(results):
    outs = []
    for i in range(NCORES):
        y = np.asarray(results[i]["yout"], dtype=np.float32)  # [128,2,16,256]
        outs.append(y.transpose(2, 1, 0, 3).reshape(BPC, 1, P, P))
    return np.ascontiguousarray(np.concatenate(outs, axis=0))


def _run(x0_pred, trace=False, tmpdir=None):
    import time
    from concourse.bass_utils import run_bass_kernel_spmd
    nc = _get_program()[0]
    in_maps = _shard_inputs(x0_pred)
    try:
        res = run_bass_kernel_spmd(nc, in_maps, list(range(NCORES)),
                                   trace=trace, tmpdir=tmpdir)
    except Exception:
        # transient NRT execution failures have been observed; one retry
        time.sleep(2.0)
        res = run_bass_kernel_spmd(nc, in_maps, list(range(NCORES)),
                                   trace=trace, tmpdir=tmpdir)
    return _unshard(res.results), res


def kernel(x0_pred):
    out, _ = _run(x0_pred, trace=False)
    return out



# revision 70
# speedup vs baseline: 1.2576x; 1.0746x over previous
"""Trainium2 Bass kernel for nn_DarcyResidual (P=256, B=128, 8 NeuronCores).

Math (reference):
    a = (x0 + 1.5) / 0.2,  p = (x1 + 0.9) / 115
    residual = -a*(p_d00 + p_d11) - a_d0*p_d0 - a_d1*p_d1 - 1
2nd-order central differences inside, 2nd-order one-sided at borders,
h = 1/256 on both axes.

Folded form computed here (G = 5/(460 h^2)):
    residual = -G * [ (X0 + 1.5)*U4 + S1*R1 + C1a*C1p ] - 1
      U4  = 4*(rowD2raw(X1) + colD2raw(X1))   (raw h^2-scaled 2nd diffs)
      R1  = rowD1raw(X1), S1 = rowD1raw(X0)   (raw 2h-scaled 1st diffs)
      C1p = colD1raw(X1), C1a = colD1raw(X0)

v2 layout per core (16 images): SBUF [partition = row-within-128-block,
free = (row-block k:2, image b, col j:256)], 8 chunks of 2 images.
All row-direction (d0) stencils are bf16 TensorE matmuls (banded stencil
matrices as lhsT blocks); the column Laplacian rides the same PSUM
accumulation via 4I matmuls on +-1-column-shifted rhs views of a padded
bf16 x1 tile.  S1*R1 is a DVE stt directly from PSUM (shifted bf16 out);
C1a*C1p comes from shifted-aligned 2x bf16 DVE stencils.  The three
terms are summed in a PSUM "res" bank by two identity bf16 matmuls
(rhs = tm and the shifted gradient sum), and ScalarE does the single
fused evacuate+affine(-G,-1)+bf16-cast.  Border columns j=0,255 keep
the f32r edge pipeline fed by a host-pregathered edge tensor.  Output
is bf16 (upcast on host); output DMAs ride the GPSIMD (SWDGE) queue.
"""

import numpy as np

P = 256
B = 128
NCORES = 8
BPC = B // NCORES          # images per core = 16
CHUNKS = 8
BCH = BPC // CHUNKS        # images per chunk = 2
FCH = 2 * BCH * P          # chunk free size = 1024
GAMMA = 5.0 * 65536.0 / 460.0

_cache = {}


def _stencils():
    D1 = np.zeros((P, P), dtype=np.float64)
    for i in range(1, P - 1):
        D1[i, i - 1] = -1.0
        D1[i, i + 1] = 1.0
    D1[0, 0:3] = [-3.0, 4.0, -1.0]
    D1[P - 1, P - 3:P] = [1.0, -4.0, 3.0]

    D2 = np.zeros((P, P), dtype=np.float64)
    for i in range(1, P - 1):
        D2[i, i - 1] = 1.0
        D2[i, i] = -2.0
        D2[i, i + 1] = 1.0
    D2[0, 0:4] = [2.0, -5.0, 4.0, -1.0]
    D2[P - 1, P - 4:P] = [-1.0, 4.0, -5.0, 2.0]
    return D1, D2


def _weights_bf16():
    """[128, 14, 128] bf16 lhsT blocks for all matmuls.
    0-3: D1 blocks; 4-7: 4*(D2-2I) blocks; 8: 4I; 9: I;
    10-13: 4*D2 blocks (edge pipeline, col stencil complete).
    All entries are small integers -> exact in bf16."""
    import ml_dtypes
    D1, D2 = _stencils()
    WR2 = 4.0 * (D2 - 2.0 * np.eye(P))
    WR2E = 4.0 * D2
    wtb = np.zeros((128, 14, 128), dtype=np.float64)
    for m in range(2):
        for kb in range(2):
            blk = lambda W: W[m * 128:(m + 1) * 128, kb * 128:(kb + 1) * 128].T
            wtb[:, m * 2 + kb, :] = blk(D1)
            wtb[:, 4 + m * 2 + kb, :] = blk(WR2)
            wtb[:, 10 + m * 2 + kb, :] = blk(WR2E)
    wtb[:, 8, :] = 4.0 * np.eye(128)
    wtb[:, 9, :] = np.eye(128)
    return wtb.astype(ml_dtypes.bfloat16)


def _build_program():
    from concourse import bacc
    import concourse.mybir as mybir
    from concourse.tile import TileContext

    f32 = mybir.dt.float32
    f32r = mybir.dt.float32r
    bf16 = mybir.dt.bfloat16
    ADD = mybir.AluOpType.add
    SUB = mybir.AluOpType.subtract
    MUL = mybir.AluOpType.mult
    COPY = mybir.ActivationFunctionType.Copy

    nc = bacc.Bacc("TRN2", target_bir_lowering=False, debug=False,
                   num_devices=NCORES)
    xe = nc.dram_tensor("xe", [128, 2, 2, BPC, 8], bf16, kind="ExternalInput")
    xb = nc.dram_tensor("xb", [128, 2, 2, BPC, P], bf16, kind="ExternalInput")
    wtbd = nc.dram_tensor("wtbd", [128, 14, 128], bf16, kind="ExternalInput")
    yout = nc.dram_tensor("yout", [128, 2, BPC, P], bf16, kind="ExternalOutput")

    with TileContext(nc) as tc:
        with (
            tc.tile_pool(name="const", bufs=1) as cpool,
            tc.tile_pool(name="edge", bufs=1) as epool,
            tc.tile_pool(name="work", bufs=2) as pool,
            tc.tile_pool(name="psum", bufs=2, space="PSUM") as pp,
        ):
            # chunk-0 bf16 inputs first, then the small weight tensors.
            # x1 goes into a 2-left-padded tile so the +-1-column-shifted
            # identity-matmul rhs views exist and the C1p stencil views
            # stay 4-byte aligned (2x mode).
            wtb = cpool.tile([128, 14, 128], bf16)
            nc.sync.dma_start(out=wtb[:], in_=wtbd[:])
            Xp0 = pool.tile([128, FCH + 4], bf16, tag="x1", bufs=3)
            nc.sync.dma_start(
                out=Xp0[:, 2:FCH + 2].rearrange(
                    "p (k b j) -> p k b j", k=2, b=BCH),
                in_=xb[:, 1, :, 0:BCH, :])
            X0c0 = pool.tile([128, 2, BCH, P], bf16, tag="x0", bufs=3)
            nc.sync.dma_start(out=X0c0[:], in_=xb[:, 0, :, 0:BCH, :])

            def Wb(i):
                return wtb[:, i, :]

            stt = nc.vector.scalar_tensor_tensor

            def emit_edge():
                # ------------- edge pipeline (output cols j=0 and j=255) -------
                X0e = epool.tile([128, 2, BPC, 8], bf16)
                X1e = epool.tile([128, 2, BPC, 8], bf16)
                nc.sync.dma_start(out=X0e[:], in_=xe[:, 0])
                nc.sync.dma_start(out=X1e[:], in_=xe[:, 1])

                X0ef = X0e.rearrange("p k b c -> p (k b c)")
                X1ef = X1e.rearrange("p k b c -> p (k b c)")
                # [128, 32, 8] views
                E1 = X1e.rearrange("p k b c -> p (k b) c")
                E0 = X0e.rearrange("p k b c -> p (k b) c")

                def et(name, d=2):
                    return epool.tile([128, 2 * BPC, d], f32, name=name, tag=name)

                if True:
                    R2e = pp.tile([128, 2, BPC, 8], f32, tag="r2")
                    R1e = pp.tile([128, 2, BPC, 8], f32, tag="r1")
                    S1e = pp.tile([128, 2, BPC, 8], f32, tag="s1")
                    R2ef = R2e.rearrange("p k b c -> p (k b c)")
                    R1ef = R1e.rearrange("p k b c -> p (k b c)")
                    S1ef = S1e.rearrange("p k b c -> p (k b c)")
                    for m in range(2):
                        osl = slice(m * 128, (m + 1) * 128)
                        for kb in range(2):
                            isl = slice(kb * 128, (kb + 1) * 128)
                            st, sp = kb == 0, kb == 1
                            nc.tensor.matmul(R1ef[:, osl], Wb(m * 2 + kb),
                                             X1ef[:, isl], start=st, stop=sp)
                            nc.tensor.matmul(S1ef[:, osl], Wb(m * 2 + kb),
                                             X0ef[:, isl], start=st, stop=sp)
                            nc.tensor.matmul(R2ef[:, osl], Wb(10 + m * 2 + kb),
                                             X1ef[:, isl], start=st, stop=sp)

                    # paired forward/mirrored diffs: half 0 = j=0 side (fwd),
                    # half 1 = j=255 side (also forward-oriented: f7-f6 etc.)
                    a1, b1, c1 = et("a1"), et("b1"), et("c1")
                    a0, b0 = et("a0"), et("b0")
                    nc.vector.tensor_sub(a1[:], E1[:, :, 1:8:6], E1[:, :, 0:7:6])
                    nc.vector.tensor_sub(b1[:], E1[:, :, 2:7:4], E1[:, :, 1:6:4])
                    nc.vector.tensor_sub(c1[:], E1[:, :, 3:6:2], E1[:, :, 2:5:2])
                    nc.vector.tensor_sub(a0[:], E0[:, :, 1:8:6], E0[:, :, 0:7:6])
                    nc.vector.tensor_sub(b0[:], E0[:, :, 2:7:4], E0[:, :, 1:6:4])

                    # one-sided raw stencils (Z sign flips on the mirror half)
                    q, Z = et("q"), et("Z")
                    C1pe, C1ae = et("C1pe"), et("C1ae")
                    stt(q[:], b1[:], 3.0, c1[:], MUL, SUB)      # 3b - c
                    stt(Z[:], a1[:], -2.0, q[:], MUL, ADD)      # -2a + 3b - c
                    stt(C1pe[:], a1[:], 3.0, b1[:], MUL, SUB)   # 3a - b
                    stt(C1ae[:], a0[:], 3.0, b0[:], MUL, SUB)

                    RP2 = R2e.rearrange("p k b c -> p (k b) c")
                    RP1 = R1e.rearrange("p k b c -> p (k b) c")
                    U4e, tme, t2e = et("U4e"), et("tme"), et("t2e")
                    stt(U4e[:, :, 0:1], Z[:, :, 0:1], 4.0, RP2[:, :, 0:1], MUL, ADD)
                    stt(U4e[:, :, 1:2], Z[:, :, 1:2], -4.0, RP2[:, :, 7:8], MUL, ADD)

                    Scpe = epool.tile([128, 2, BPC, 8], f32)
                    nc.scalar.copy(out=Scpe.rearrange("p k b c -> p (k b c)"),
                                   in_=S1ef[:])
                    SP = Scpe.rearrange("p k b c -> p (k b) c")

                    stt(tme[:], E0[:, :, 0:8:7], 1.5, U4e[:], ADD, MUL)
                    nc.vector.tensor_mul(t2e[:], SP[:, :, 0:8:7], RP1[:, :, 0:8:7])
                    nc.vector.tensor_add(tme[:], tme[:], t2e[:])
                    nc.vector.tensor_mul(C1ae[:], C1ae[:], C1pe[:])  # t3e in-place
                    nc.vector.tensor_add(tme[:], tme[:], C1ae[:])
                    rese = epool.tile([128, 2, BPC, 2], f32)
                    nc.scalar.activation(
                        rese.rearrange("p k b e -> p (k b) e"), tme[:], COPY,
                        bias=-1.0, scale=-GAMMA)

                return rese

            # ------------- main pipeline, 8 chunks of 2 images -------------
            if True:
                for c in range(CHUNKS):
                    b0c = c * BCH
                    if c == 0:
                        X0c, Xp = X0c0, Xp0
                    else:
                        X0c = pool.tile([128, 2, BCH, P], bf16, tag="x0",
                                        bufs=3)
                        Xp = pool.tile([128, FCH + 4], bf16, tag="x1",
                                       bufs=3)
                        nc.sync.dma_start(
                            out=Xp[:, 2:FCH + 2].rearrange(
                                "p (k b j) -> p k b j", k=2, b=BCH),
                            in_=xb[:, 1, :, b0c:b0c + BCH, :])
                        nc.sync.dma_start(out=X0c[:],
                                          in_=xb[:, 0, :, b0c:b0c + BCH, :])
                    X0f = X0c.rearrange("p k b j -> p (k b j)")
                    X1f = Xp[:, 2:FCH + 2]
                    C1p = pool.tile([128, FCH], bf16, tag="c1p", bufs=3)
                    C1a = pool.tile([128, FCH], bf16, tag="c1a", bufs=3)
                    t3b = pool.tile([128, FCH], bf16, tag="t3b", bufs=3)
                    t2b = pool.tile([128, FCH], bf16, tag="t2b", bufs=3)
                    rcp = pool.tile([128, FCH], bf16, tag="rcp", bufs=3)
                    u4b = pool.tile([128, FCH], bf16, tag="u4b", bufs=3)
                    tm = pool.tile([128, 2, BCH, P], bf16, tag="tm", bufs=3)
                    tmf = tm.rearrange("p k b j -> p (k b j)")

                    # column stencils, shifted layout (slot t = col t+1),
                    # all views 4-byte aligned -> 2x mode
                    nc.vector.tensor_sub(C1p[:, 0:FCH - 2], Xp[:, 4:FCH + 2],
                                         Xp[:, 2:FCH])
                    nc.vector.tensor_sub(C1a[:, 0:FCH - 2], X0f[:, 2:FCH],
                                         X0f[:, 0:FCH - 2])
                    nc.vector.tensor_mul(t3b[:], C1a[:], C1p[:])

                    res = pp.tile([128, 2, BCH, P], f32, name=f"res_{c}",
                                  tag="res", bufs=1)
                    resf = res.rearrange("p k b j -> p (k b j)")
                    for m in range(2):
                        R1s = pp.tile([128, 2 * P], f32, name=f"r1_{c}_{m}",
                                      tag="r1")
                        S1s = pp.tile([128, 2 * P], f32, name=f"s1_{c}_{m}",
                                      tag="s1")
                        U4s = pp.tile([128, 2 * P], f32, name=f"r2_{c}_{m}",
                                      tag="r2")
                        for kb in range(2):
                            st, sp = kb == 0, kb == 1
                            isl = slice(kb * (BCH * P), (kb + 1) * (BCH * P))
                            nc.tensor.matmul(R1s[:], Wb(m * 2 + kb),
                                             X1f[:, isl], start=st, stop=sp)
                            nc.tensor.matmul(S1s[:], Wb(m * 2 + kb),
                                             X0f[:, isl], start=st, stop=sp)
                            nc.tensor.matmul(U4s[:], Wb(4 + m * 2 + kb),
                                             X1f[:, isl], start=st, stop=False)
                        # column-neighbor sums via 4I with +-1-shifted rhs:
                        # U4 = W_R2@X1 + 4I@X1[+1] + 4I@X1[-1], all in PSUM
                        lo = m * (BCH * P)
                        hi = lo + 2 * P
                        nc.tensor.matmul(U4s[:], Wb(8),
                                         Xp[:, lo + 3:hi + 3],
                                         start=False, stop=False)
                        nc.tensor.matmul(U4s[:], Wb(8),
                                         Xp[:, lo + 1:hi + 1],
                                         start=False, stop=True)
                        # DVE can read only one PSUM operand per op: ScalarE
                        # evacuates R1 (shifted bf16), S1 stays in PSUM.
                        nc.scalar.copy(out=rcp[:, lo:hi - 1],
                                       in_=R1s[:, 1:2 * P])
                        # t2b slot t = S1*R1 at col t+1 (shifted bf16 out)
                        stt(t2b[:, lo:hi - 1], S1s[:, 1:2 * P], 1.0,
                            rcp[:, lo:hi - 1], MUL, MUL)
                        # ScalarE evacuates U4 so the tm product runs all-bf16
                        # SBUF 4B-aligned -> DVE 2x mode
                        nc.scalar.copy(out=u4b[:, lo:hi], in_=U4s[:])
                        # tm = X0' * U4 (bf16 out); the +1.5 is folded into
                        # the host input (all stencil row sums are zero so
                        # S1/C1a are unchanged), making this a plain TT that
                        # actually gets the 2x mode (stt is always 1x)
                        nc.vector.tensor_mul(tmf[:, lo:hi], X0f[:, lo:hi],
                                             u4b[:, lo:hi])

                    # gradient-product sum in shifted bf16 (2x), then the
                    # whole merge happens on TensorE in the res PSUM bank
                    nc.vector.tensor_add(t2b[:], t2b[:], t3b[:])
                    # PSUM-bank-sized (N<=512) merge matmuls
                    H = FCH // 2
                    nc.tensor.matmul(resf[:, 1:H], Wb(9), tmf[:, 1:H],
                                     start=True, stop=False)
                    nc.tensor.matmul(resf[:, 1:H], Wb(9), t2b[:, 0:H - 1],
                                     start=False, stop=True)
                    nc.tensor.matmul(resf[:, H:FCH], Wb(9), tmf[:, H:FCH],
                                     start=True, stop=False)
                    nc.tensor.matmul(resf[:, H:FCH], Wb(9),
                                     t2b[:, H - 1:FCH - 1],
                                     start=False, stop=True)

                    if c == 0:
                        rese = emit_edge()
                    outt = pool.tile([128, 2, BCH, P], bf16, tag="out", bufs=3)
                    nc.scalar.activation(outt[:, :, :, 1:P - 1],
                                         res[:, :, :, 1:P - 1], COPY,
                                         bias=-1.0, scale=-GAMMA)
                    # edge columns j=0,255 from the edge pipeline (one copy)
                    nc.scalar.copy(out=outt[:, :, :, 0:P:P - 1],
                                   in_=rese[:, :, b0c:b0c + BCH, :])
                    nc.gpsimd.dma_start(
                        out=yout[:, :, b0c:b0c + BCH, :], in_=outt[:])

    nc.compile()
    return nc


def _get_program():
    if "nc" not in _cache:
        _cache["nc"] = _build_program()
        _cache["wtbd"] = _weights_bf16()
    return _cache["nc"], _cache["wtbd"]


def _shard_inputs(x0_pred):
    import ml_dtypes
    x = np.ascontiguousarray(np.asarray(x0_pred, dtype=np.float32))
    _, wtbd = _get_program()
    in_maps = []
    for i in range(NCORES):
        shard = x[i * BPC:(i + 1) * BPC]                      # [16,2,256,256]
        arr = shard.reshape(BPC, 2, 2, 128, P).transpose(3, 1, 2, 0, 4)
        cols = [0, 1, 2, 3, P - 4, P - 3, P - 2, P - 1]
        # edge pipeline keeps RAW x0 (it applies +1.5 itself)
        xe = np.ascontiguousarray(arr[:, :, :, :, cols]).astype(
            ml_dtypes.bfloat16)
        # main pipeline gets X0' = x0 + 1.5 so tm is a plain product
        arrp = arr.copy()
        arrp[:, 0] += 1.5
        xbi = np.ascontiguousarray(arrp).astype(ml_dtypes.bfloat16)
        in_maps.append({"xe": xe, "xb": xbi, "wtbd": wtbd})
    return in_maps


def _unshard(results):
    outs = []
    for i in range(NCORES):
        y = np.asarray(results[i]["yout"], dtype=np.float32)  # [128,2,16,256]
        outs.append(y.transpose(2, 1, 0, 3).reshape(BPC, 1, P, P))
    return np.ascontiguousarray(np.concatenate(outs, axis=0))


def _run(x0_pred, trace=False, tmpdir=None):
    import time
    from concourse.bass_utils import run_bass_kernel_spmd
    nc = _get_program()[0]
    in_maps = _shard_inputs(x0_pred)
    try:
        res = run_bass_kernel_spmd(nc, in_maps, list(range(NCORES)),
                                   trace=trace, tmpdir=tmpdir)
    except Exception:
        # transient NRT execution failures have been observed; one retry
        time.sleep(2.0)
        res = run_bass_kernel_spmd(nc, in_maps, list(range(NCORES)),
                                   trace=trace, tmpdir=tmpdir)
    return _unshard(res.results), res


def kernel(x0_pred):
    out, _ = _run(x0_pred, trace=False)
    return out

